# revision 15
# baseline (speedup 1.0000x reference)
"""2-layer GraphSAGE (mean agg) + two linear heads on 8 Trainium2 NeuronCores.

Strategy (dst-sharded data parallel):
- Nodes are padded 100000 -> 100352 = 8*12544 and sharded contiguously: core c
  owns dst rows [c*12544, (c+1)*12544) (last 44 rows of each shard are padding).
- Edges are routed to the core owning their dst, grouped into 98 dst tiles of
  128 nodes, split into chunks of 128 edges (chunk counts uniform across cores;
  padding lanes use src=0 / dstloc=300 which contribute nothing).
- Per chunk: indirect DMA gathers 128 table rows h[src] -> [128e, 128f] fp32;
  DVE scales by 1/deg(dst) and casts to bf16; DVE builds the one-hot selection
  matrix sel[e,d] = (dstloc[e]==d); PE accumulates accT[f,d] += msg^T @ sel in
  PSUM over the tile's chunks.  accT is the transposed mean-aggregation.
- Layer GEMMs run from accT (already transposed) + a PE transpose of the own
  rows; layer 1 writes h1 (fp32) to its shard, an ncfw AllGather assembles the
  full h1 table for layer 2's gather; layer 2 emits the two heads directly.
"""
import sys
import time

sys.path.insert(0, "/opt/trn_rl_repo")

import numpy as np

N_NODES = 100000
D = 128
NCORE = 8
SH = 12500            # real nodes per core
SHP = 12544           # padded nodes per core (98 * 128)
NP = NCORE * SHP      # padded node count 100352
T = SHP // 128        # dst tiles per core (98)
PAD_DST = 300.0       # dstloc value for padding lanes (no sel match)

_cache = {}


def _tilefix():
    """Walrus in this env supports only one sync-wait command per instruction.
    Split Tile's fat kernel-tail drain and any multi-wait instruction into
    single-wait NOP chains."""
    import concourse.tile as tile
    import concourse.mybir as mybir
    import bass_rust

    if getattr(tile.TileContext, "_gnn_tilefix", False):
        return
    orig_schedule = tile.TileContext.schedule_and_allocate
    uid = [0]

    def mk_nop(engine, waits):
        uid[0] += 1
        nop = mybir.InstNoOp(name=f"waitnop-{uid[0]}", ins=[], outs=[])
        nop.engine = engine
        nop.sync_info = mybir.SyncInfo(on_wait=list(waits), on_update=[])
        return nop

    def split_multiwaits(nc):
        for f in nc.m.functions:
            for bb in f.blocks:
                out, changed = [], False
                for inst in bb.instructions:
                    si = inst.sync_info
                    if si is not None and len(si.on_wait) > 1:
                        waits = list(si.on_wait)
                        for w in waits[:-1]:
                            out.append(mk_nop(inst.engine, [w]))
                        inst.sync_info = mybir.SyncInfo(
                            on_wait=[waits[-1]], on_update=list(si.on_update)
                        )
                        changed = True
                    out.append(inst)
                if changed:
                    bb.instructions = out

    def drain_and_barrier(self, tick_clock, wait_clock):
        nop0 = self.nc.sync.nop(nofuse=True)
        wait_clock.add_sem_waits(
            nop0.ins, bass_rust.ScopedClock({None: tick_clock.global_clock})
        )
        self.nc.all_engine_barrier()
        assert self.sems is not None
        popped = self.nc._tile_sem_poison_stack.pop()
        assert popped is self._sem_poison
        self.nc.clear_and_free_semaphores(list(self.sems.allocated().values()))
        self.nc.all_engine_barrier()

    def schedule_and_allocate(self, *a, **kw):
        res = orig_schedule(self, *a, **kw)
        split_multiwaits(self.nc)
        return res

    tile.TileContext._drain_and_barrier = drain_and_barrier
    tile.TileContext.schedule_and_allocate = schedule_and_allocate
    tile.TileContext._gnn_tilefix = True


def _program(nch):
    """Build the Bass program. nch[t] = chunk count of dst tile t (uniform
    across cores)."""
    import concourse.bass as bass
    import concourse.tile as tile
    import concourse.mybir as mybir

    _tilefix()
    f32, bf16, i32 = mybir.dt.float32, mybir.dt.bfloat16, mybir.dt.int32
    P = 128
    totch = sum(nch)

    nc = bass.Bass(num_devices=NCORE)
    x_own = nc.declare_dram_parameter("x_own", [SHP, D], bf16, isOutput=False)
    idx_in = nc.declare_dram_parameter("idx", [P, totch], i32, isOutput=False)
    dstloc_in = nc.declare_dram_parameter("dstloc", [P, totch], bf16, isOutput=False)
    wedge_in = nc.declare_dram_parameter("wedge", [P, totch], bf16, isOutput=False)
    iota_in = nc.declare_dram_parameter("iotac", [P, P], bf16, isOutput=False)
    w1l_in = nc.declare_dram_parameter("w1l", [D, D], bf16, isOutput=False)
    w1r_in = nc.declare_dram_parameter("w1r", [D, D], bf16, isOutput=False)
    w2l_in = nc.declare_dram_parameter("w2l", [D, D], bf16, isOutput=False)
    w2r_in = nc.declare_dram_parameter("w2r", [D, D], bf16, isOutput=False)
    wpd_in = nc.declare_dram_parameter("wpd", [D, D], bf16, isOutput=False)
    lo_out = nc.declare_dram_parameter("lo", [P, P], f32, isOutput=True)
    hi_out = nc.declare_dram_parameter("hi", [P, P], f32, isOutput=True)

    x_own_b = nc.dram_tensor("x_own_b", [SHP, D], bf16)
    x_full = nc.dram_tensor("x_full", [NP, D], bf16)
    h1_shard = nc.dram_tensor("h1_shard", [SHP, D], bf16)
    h1_full = nc.dram_tensor("h1_full", [NP, D], bf16)

    from concourse.masks import make_identity

    with tile.TileContext(nc) as tc:
        with (
            tc.tile_pool(name="stage", bufs=1) as stage,
            tc.tile_pool(name="gb", bufs=12) as gbp,
            tc.tile_pool(name="work", bufs=4) as work,
            tc.tile_pool(name="acps", bufs=2, space="PSUM") as acps,
            tc.tile_pool(name="wkps", bufs=2, space="PSUM") as wkps,
        ):
            idx_t = stage.tile([P, totch], i32)
            nc.sync.dma_start(out=idx_t[:], in_=idx_in[:])
            dstloc_t = stage.tile([P, totch], bf16)
            nc.sync.dma_start(out=dstloc_t[:], in_=dstloc_in[:])
            wedge_t = stage.tile([P, totch], bf16)
            nc.sync.dma_start(out=wedge_t[:], in_=wedge_in[:])
            iota_t = stage.tile([P, P], bf16)
            nc.sync.dma_start(out=iota_t[:], in_=iota_in[:])
            w1l = stage.tile([D, D], bf16)
            nc.sync.dma_start(out=w1l[:], in_=w1l_in[:])
            w1r = stage.tile([D, D], bf16)
            nc.sync.dma_start(out=w1r[:], in_=w1r_in[:])
            w2l = stage.tile([D, D], bf16)
            nc.sync.dma_start(out=w2l[:], in_=w2l_in[:])
            w2r = stage.tile([D, D], bf16)
            nc.sync.dma_start(out=w2r[:], in_=w2r_in[:])
            wpd = stage.tile([D, D], bf16)
            nc.sync.dma_start(out=wpd[:], in_=wpd_in[:])
            ident = stage.tile([P, P], f32)
            make_identity(nc, ident[:])
            ident_bf = stage.tile([P, P], bf16)
            nc.vector.tensor_copy(out=ident_bf[:], in_=ident[:])

            # assemble the full x table on device (saves host->device upload)
            nc.sync.dma_start(out=x_own_b[:], in_=x_own[:])
            nc.gpsimd.collective_compute(
                "AllGather", mybir.AluOpType.bypass,
                replica_groups=[list(range(NCORE))],
                ins=[x_own_b[:]], outs=[x_full[:]])

            def aggregate_tile(table, t, ch0):
                """accT[f,d] for dst tile t; returns SBUF bf16 [fin, node]."""
                accT = acps.tile([P, P], f32, space="PSUM", tag="accT")
                n = nch[t]
                for j in range(n):
                    ch = ch0 + j
                    gb = gbp.tile([P, P], bf16, tag="gb")
                    nc.gpsimd.indirect_dma_start(
                        out=gb[:], out_offset=None, in_=table[:],
                        in_offset=bass.IndirectOffsetOnAxis(
                            ap=idx_t[:, ch:ch + 1], axis=0))
                    msg = work.tile([P, P], bf16, tag="msg")
                    nc.vector.tensor_tensor(
                        out=msg[:], in0=gb[:],
                        in1=wedge_t[:, ch:ch + 1].to_broadcast([P, P]),
                        op=mybir.AluOpType.mult)
                    sel = work.tile([P, P], bf16, tag="sel")
                    nc.vector.tensor_tensor(
                        out=sel[:], in0=dstloc_t[:, ch:ch + 1].to_broadcast([P, P]),
                        in1=iota_t[:], op=mybir.AluOpType.is_equal)
                    nc.tensor.matmul(out=accT[:], lhsT=msg[:], rhs=sel[:],
                                     start=(j == 0), stop=(j == n - 1))
                aggT = work.tile([P, P], bf16, tag="aggT")
                nc.vector.tensor_copy(out=aggT[:], in_=accT[:])
                return aggT

            def own_T(own_dram, t):
                """Own rows tile t transposed -> SBUF bf16 [fin, node]."""
                rows = work.tile([P, D], bf16, tag="ownrows")
                nc.sync.dma_start(out=rows[:], in_=own_dram[t * P:(t + 1) * P, :])
                tps = wkps.tile([P, P], bf16, space="PSUM", tag="tps")
                nc.tensor.transpose(out=tps[:], in_=rows[:], identity=ident_bf[:])
                hT = work.tile([P, P], bf16, tag="hT")
                nc.vector.tensor_copy(out=hT[:], in_=tps[:])
                return hT

            # ---------------- layer 1 ----------------
            ch0 = 0
            for t in range(T):
                aggT = aggregate_tile(x_full, t, ch0)
                ch0 += nch[t]
                xT = own_T(x_own, t)
                yps = wkps.tile([P, P], f32, space="PSUM", tag="yps")
                nc.tensor.matmul(out=yps[:], lhsT=aggT[:], rhs=w1l[:],
                                 start=True, stop=False)
                nc.tensor.matmul(out=yps[:], lhsT=xT[:], rhs=w1r[:],
                                 start=False, stop=True)
                h1t = work.tile([P, P], bf16, tag="h1t")
                nc.vector.tensor_relu(out=h1t[:], in_=yps[:])
                nc.sync.dma_start(out=h1_shard[t * P:(t + 1) * P, :], in_=h1t[:])

            nc.gpsimd.collective_compute(
                "AllGather", mybir.AluOpType.bypass,
                replica_groups=[list(range(NCORE))],
                ins=[h1_shard[:]], outs=[h1_full[:]])

            # ---------------- layer 2 + heads ----------------
            ch0 = 0
            for t in range(T):
                aggT = aggregate_tile(h1_full, t, ch0)
                ch0 += nch[t]
                hT = own_T(h1_shard, t)
                yps = wkps.tile([P, P], f32, space="PSUM", tag="yps")
                # YT[fo, node] = W2l^T @ aggT + W2r^T @ h1T
                nc.tensor.matmul(out=yps[:], lhsT=w2l[:], rhs=aggT[:],
                                 start=True, stop=False)
                nc.tensor.matmul(out=yps[:], lhsT=w2r[:], rhs=hT[:],
                                 start=False, stop=True)
                h2T = work.tile([P, P], bf16, tag="h2T")
                nc.vector.tensor_relu(out=h2T[:], in_=yps[:])
                p_ps = wkps.tile([1, P], f32, space="PSUM", tag="pd")
                nc.tensor.matmul(out=p_ps[:], lhsT=wpd[:, 0:1], rhs=h2T[:],
                                 start=True, stop=True)
                d_ps = wkps.tile([1, P], f32, space="PSUM", tag="pd")
                nc.tensor.matmul(out=d_ps[:], lhsT=wpd[:, 1:2], rhs=h2T[:],
                                 start=True, stop=True)
                sig = work.tile([1, P], f32, tag="sig")
                nc.scalar.activation(out=sig[:], in_=d_ps[:],
                                     func=mybir.ActivationFunctionType.Sigmoid)
                pr = work.tile([1, P], f32, tag="pr")
                nc.vector.tensor_copy(out=pr[:], in_=p_ps[:])
                lo_t = work.tile([1, P], f32, tag="lot")
                nc.vector.tensor_sub(out=lo_t[:], in0=pr[:], in1=sig[:])
                hi_t = work.tile([1, P], f32, tag="hit")
                nc.vector.tensor_add(out=hi_t[:], in0=pr[:], in1=sig[:])
                nc.sync.dma_start(out=lo_out[t:t + 1, :], in_=lo_t[:])
                nc.sync.dma_start(out=hi_out[t:t + 1, :], in_=hi_t[:])

    return nc


def _preprocess(inputs):
    import ml_dtypes

    x = np.asarray(inputs["x"], dtype=np.float32)
    ei = np.asarray(inputs["edge_index"])
    src = np.asarray(ei[0], dtype=np.int64)
    dst = np.asarray(ei[1], dtype=np.int64)
    n = x.shape[0]
    assert n == N_NODES

    deg = np.bincount(dst, minlength=n).astype(np.float32)
    inv_deg = 1.0 / np.maximum(deg, 1.0)

    srcp = (src // SH) * SHP + (src % SH)          # padded src ids
    core = dst // SH
    dloc = dst % SH                                 # 0..12499 within shard
    tl = dloc // 128                                # dst tile
    lane = dloc % 128

    # per (core, tile) edge lists
    order = np.lexsort((tl, core))
    srcp_s, core_s, tl_s, lane_s, w_s = (
        srcp[order], core[order], tl[order], lane[order], inv_deg[dst[order]])
    # counts [NCORE, T]
    cnt = np.zeros((NCORE, T), dtype=np.int64)
    np.add.at(cnt, (core_s, tl_s), 1)
    nch = np.maximum(1, ((cnt.max(axis=0) + 127) // 128)).astype(np.int64)
    totch = int(nch.sum())

    # slot base per (core, tile)
    tile_base = np.concatenate([[0], np.cumsum(nch)])[:-1] * 128  # [T]
    idx_arr = np.zeros((NCORE, 128, totch), dtype=np.int32)
    dst_arr = np.full((NCORE, 128, totch), PAD_DST, dtype=np.float32)
    wdg_arr = np.zeros((NCORE, 128, totch), dtype=np.float32)

    # positions of each edge within its (core, tile) bucket
    # edges sorted by (core, tile): within-bucket rank
    bucket_start = np.zeros((NCORE, T), dtype=np.int64)
    flat_cnt = cnt.ravel()
    starts = np.concatenate([[0], np.cumsum(flat_cnt)])[:-1]
    bucket_start = starts.reshape(NCORE, T)
    pos_in_bucket = np.arange(len(srcp_s)) - bucket_start[core_s, tl_s]
    slot = tile_base[tl_s] + pos_in_bucket          # slot within core's schedule
    p_lane = slot % 128
    chn = slot // 128
    idx_arr[core_s, p_lane, chn] = srcp_s
    dst_arr[core_s, p_lane, chn] = lane_s
    wdg_arr[core_s, p_lane, chn] = w_s

    iota = np.tile(np.arange(128, dtype=np.float32), (128, 1))

    bf = ml_dtypes.bfloat16
    wmats = {k: np.asarray(inputs[k], dtype=np.float32) for k in
             ("W1l", "W1r", "W2l", "W2r", "Wp", "Wd")}
    wpd = np.zeros((D, D), dtype=np.float32)
    wpd[:, 0:1] = wmats["Wp"]
    wpd[:, 1:2] = wmats["Wd"]

    in_maps = []
    for c in range(NCORE):
        x_own_c = np.zeros((SHP, D), dtype=np.float32)
        x_own_c[:SH] = x[c * SH:(c + 1) * SH]
        in_maps.append({
            "x_own": x_own_c.astype(bf),
            "idx": idx_arr[c],
            "dstloc": dst_arr[c].astype(bf),
            "wedge": wdg_arr[c].astype(bf),
            "iotac": iota.astype(bf),
            "w1l": wmats["W1l"].astype(bf),
            "w1r": wmats["W1r"].astype(bf),
            "w2l": wmats["W2l"].astype(bf),
            "w2r": wmats["W2r"].astype(bf),
            "wpd": wpd.astype(bf),
        })
    return in_maps, [int(v) for v in nch]


def kernel(**inputs):
    from concourse.bass_utils import run_bass_kernel_spmd

    in_maps, nch = _preprocess(inputs)
    key = tuple(nch)
    if key not in _cache:
        _cache[key] = _program(nch)
    nc = _cache[key]

    t0 = time.perf_counter()
    res = run_bass_kernel_spmd(nc, in_maps, core_ids=list(range(NCORE)))
    t1 = time.perf_counter()
    kernel.last_exec_wall_s = t1 - t0

    lo = np.empty((N_NODES, 1), dtype=np.float32)
    hi = np.empty((N_NODES, 1), dtype=np.float32)
    for c in range(NCORE):
        lo[c * SH:(c + 1) * SH, 0] = res.results[c]["lo"].reshape(-1)[:SH]
        hi[c * SH:(c + 1) * SH, 0] = res.results[c]["hi"].reshape(-1)[:SH]
    return lo, hi
